# revision 27
# baseline (speedup 1.0000x reference)
"""GraphSAGE 2-layer mini-batch kernel for 8 Trainium2 NeuronCores (v3).

Strategy: data-parallel over the batch (128 targets per core); the feature
table is uploaded as fp16 (halves gather bytes; tolerance is 2e-2).

ALL 36,608 rows per core (nb1_self, nb1_nb, nodes, nb2) go through the Q7
dma_gather path: indices bucket-sorted into 16 buckets of 32768 rows
(int16-addressable), one dma_gather per bucket over 4 SWDGE queues.
nodes/nb2 rows are singleton-group "chains" 11..21 so the same
selection-matmul machinery that undoes the bucket permutation also lands
them transposed [feature, row] — no separate indirect DMAs / PE transposes.

v3 changes vs v2:
- sel matrices are built in [row, window, entry] layout (entry innermost,
  step-1 on every operand) so the DVE runs in 2x_1p packed mode — the v2
  layout broadcast grp along the innermost axis which forced 1x.
- the agg matmuls read sel with a strided rhs AP (entry-stride columns).
- one dma_gather per bucket (16 calls instead of 20) to cut fixed SWDGE
  overhead; descriptor generation on the Q7 is the serial bottleneck.
- the whole SAGE phase runs in fp16 (weights, activations, norms): fp16
  matmuls stream 1 col/cycle vs fp32r's 4-cycle penalty, and the final
  normalize multiply runs packed.
- the per-head reciprocal runs on the tiny [1,128] norm vector BEFORE the
  rank-1 broadcast matmul (v2 reciprocal'd the broadcast [128,128]).

The 1/25 and 1/10 mean scalings are folded into host-prescaled W1/W2.
L2 norms use a ones-vector matmul for the cross-partition reduction.
"""
import sys

sys.path.insert(0, "/opt/trn_rl_repo")

import numpy as np

P = 128
D = 128
B = 1024
S1 = 25
S2 = 10
N_NODES = 500000
NCORES = 8
B_LOC = B // NCORES          # 128 targets per core
NCHAIN_AGG = 11              # nb1_self + 10 nb1_nb chains (mean groups)
NCHAIN = 22                  # + nodes chain + 10 nb2 chains (singletons)
BUCKET_BITS = 15
BUCKET = 1 << BUCKET_BITS    # 32768 rows per bucket (int16 addressable)
NBUK = (N_NODES + BUCKET - 1) // BUCKET  # 16
NQ = 4                       # SWDGE queues: 4 rings = 4x descgen core-pairs
                             # and 4x outstanding SDMA descriptors
W_SEL = 192                  # sel window width (covers a tile's group span)
K_SEL = 16                   # sel entries built per DVE op
NGRP = NCHAIN * B_LOC        # 2816 global groups
SINGLE_PACKET = True         # concatenate each gather's descs per engine:
                             # amortizes per-packet SDMA overhead/latency
SCRATCH_SIZE = 131072       # SWDGE descriptor carveout (per-partition bytes)


def _prep_indices(nodes, nb2, nb1_self, nb1_nb):
    """Bucket-sort all gather indices per core; build device-side arrays and
    the (core-independent) per-entry metadata.

    Entry = (tile, base_chain): a 256-wide sel window covering chains
    {base, base+1}.  Tiles spanning more than 2 chains get several entries.
    """
    per_core = []
    for c in range(NCORES):
        sl = slice(c * B_LOC, (c + 1) * B_LOC)
        n1s = nb1_self[sl]              # [128, 25]
        n1n = nb1_nb[sl]                # [128, 10, 25]
        idx_chains = [n1s.reshape(-1).astype(np.int64)]
        grp_chains = [np.repeat(np.arange(B_LOC, dtype=np.int64), S1)]
        for j in range(S2):
            idx_chains.append(n1n[:, j, :].reshape(-1).astype(np.int64))
            grp_chains.append((j + 1) * B_LOC
                              + np.repeat(np.arange(B_LOC, dtype=np.int64), S1))
        # singleton chains: 11 = nodes, 12+j = nb2[:, j]
        idx_chains.append(nodes[sl].astype(np.int64))
        grp_chains.append(NCHAIN_AGG * B_LOC + np.arange(B_LOC, dtype=np.int64))
        for j in range(S2):
            idx_chains.append(nb2[sl, j].astype(np.int64))
            grp_chains.append((NCHAIN_AGG + 1 + j) * B_LOC
                              + np.arange(B_LOC, dtype=np.int64))
        all_idx = np.concatenate(idx_chains)   # [36608]
        all_grp = np.concatenate(grp_chains)
        bkt = all_idx >> BUCKET_BITS
        order = np.argsort(bkt, kind="stable")
        sidx, sgrp, sbkt = all_idx[order], all_grp[order], bkt[order]
        locs, grps = [], []
        for b in range(NBUK):
            m = sbkt == b
            locs.append((sidx[m] - (b << BUCKET_BITS)).astype(np.int64))
            grps.append(sgrp[m])
        per_core.append((locs, grps))

    # consistent per-bucket tile counts across cores (SPMD: one program)
    Cb = [max((len(per_core[c][0][b]) + P - 1) // P for c in range(NCORES))
          for b in range(NBUK)]
    Cb = [max(cb, 1) for cb in Cb]
    T_total = sum(Cb)

    S_total = T_total * P
    # Padding rows: buckets 0-5 pad with row 0 (their SBUF slots are fresh —
    # unwritten garbage could be Inf/NaN and poison the 0-weighted matmul);
    # buckets 6+ pad with -1 (trailing negatives skip descgen AND drain; the
    # recycled slot holds finite fp16 from an earlier bucket).
    idx16_cores, grp_glob = [], []
    for c in range(NCORES):
        locs, grps = per_core[c]
        lidx = np.zeros(S_total, np.int64)
        lgrp = np.full(S_total, -1024.0, np.float64)
        off = 0
        for b in range(NBUK):
            n = len(locs[b])
            lidx[off:off + n] = locs[b]
            lgrp[off:off + n] = grps[b]
            off += Cb[b] * P
        wrapped_cols = []
        off = 0
        for b in range(NBUK):
            nb_pad = Cb[b] * P
            w = lidx[off:off + nb_pad].reshape(-1, 16).T.astype(np.int16)
            wrapped_cols.append(np.tile(w, (8, 1)))
            off += nb_pad
        idx16_cores.append(np.hstack(wrapped_cols))        # [128, T_total*8]
        grp_glob.append(lgrp.reshape(T_total, P).T)        # [128, T_total]

    # per-tile GROUP spans (global group ids are sorted within a bucket, so a
    # 128-row tile covers a narrow contiguous group range), unioned across
    # cores so metadata is SPMD-safe
    tile_span = []
    for t in range(T_total):
        glo, ghi = None, None
        for c in range(NCORES):
            g = grp_glob[c][:, t]
            v = g[g >= 0]
            if v.size:
                l, h = int(v.min()), int(v.max())
                glo = l if glo is None else min(glo, l)
                ghi = h if ghi is None else max(ghi, h)
        tile_span.append((glo, ghi))

    # entries: (tile, base, s0, wtot) — an unaligned W_SEL-wide sel window at
    # global-group offset `base`.  `base` is nudged down so the window never
    # crosses a 512-col PSUM bank boundary (single matmul per entry); the
    # matmul streams sel columns [s0, wtot) so overlapping windows of the
    # same tile never double-count a row.
    def pick_base(lo):
        base = lo
        if base % 512 > 512 - W_SEL:
            base = (base // 512) * 512 + (512 - W_SEL)
        return base

    tile_base = np.cumsum([0] + Cb)
    entries_by_bucket = []
    for b in range(NBUK):
        ents = []
        for tl in range(Cb[b]):
            t = tile_base[b] + tl
            glo, ghi = tile_span[t]
            if glo is None:
                continue
            nxt = glo
            first = True
            while True:
                base = pick_base(nxt)
                s0 = 0 if first else nxt - base
                wtot = min(W_SEL, NGRP - base)
                ents.append((t, base, s0, wtot))
                first = False
                nxt = base + W_SEL
                if nxt > ghi:
                    break
        entries_by_bucket.append(ents)

    # per-core grp data laid out per ENTRY, values local to the entry window.
    # Each bucket's entry-column block is padded to an even count so every
    # K_SEL-aligned chunk slice starts 4B-aligned (2x_1p DVE mode).
    Eb = [len(e) for e in entries_by_bucket]
    Eb_pad = [e + (e % 2) for e in Eb]
    bucket_ebase = np.cumsum([0] + Eb_pad)
    E_pad_total = int(bucket_ebase[-1])
    grp_ent_cores = []
    for c in range(NCORES):
        ge = np.full((P, E_pad_total), -2048.0, np.float16)
        for b in range(NBUK):
            for j, (t, base, s0, wtot) in enumerate(entries_by_bucket[b]):
                # local offsets; pad rows (-1024 global) clamp to -2048 so the
                # fp16 value stays integer-exact and never matches iota
                loc = grp_glob[c][:, t] - base
                loc[loc < -2048] = -2048
                ge[:, bucket_ebase[b] + j] = loc.astype(np.float16)
        grp_ent_cores.append(ge)

    return dict(Cb=Cb, T_total=T_total, tile_base=tile_base,
                entries_by_bucket=entries_by_bucket, Eb=Eb,
                bucket_ebase=bucket_ebase, E_pad_total=E_pad_total,
                idx16_cores=idx16_cores, grp_ent_cores=grp_ent_cores)


def _build_program(meta, trace_sim=False):
    import concourse.bacc as bacc_mod
    import concourse.tile as tile
    from concourse import mybir

    f32 = mybir.dt.float32
    f16 = mybir.dt.float16
    Cb = meta["Cb"]
    T_total = meta["T_total"]
    tile_base = meta["tile_base"]
    entries_by_bucket = meta["entries_by_bucket"]
    Eb = meta["Eb"]
    bucket_ebase = meta["bucket_ebase"]
    E_pad_total = meta["E_pad_total"]

    nc = bacc_mod.Bacc(num_swdge_queues=NQ,
                       dynamic_dma_scratch_size=SCRATCH_SIZE)

    x_d = nc.declare_dram_parameter("x16", [N_NODES, D], f16, isOutput=False)
    w1a_d = nc.declare_dram_parameter("w1a", [D, D], f16, isOutput=False)
    w1b_d = nc.declare_dram_parameter("w1b", [D, D], f16, isOutput=False)
    w2a_d = nc.declare_dram_parameter("w2a", [D, D], f16, isOutput=False)
    w2b_d = nc.declare_dram_parameter("w2b", [D, D], f16, isOutput=False)
    b1_d = nc.declare_dram_parameter("b1v", [D, 1], f32, isOutput=False)
    b2_d = nc.declare_dram_parameter("b2v", [D, 1], f32, isOutput=False)
    iota_d = nc.declare_dram_parameter("iota", [P, W_SEL * K_SEL], f16,
                                       isOutput=False)
    idx16_d = nc.declare_dram_parameter("idx16", [P, T_total * 8],
                                        mybir.dt.int16, isOutput=False)
    grp_d = nc.declare_dram_parameter("grp", [P, E_pad_total], f16,
                                      isOutput=False)
    zt_d = nc.declare_dram_parameter("zt", [D, B_LOC], f16, isOutput=True)

    with tile.TileContext(nc, trace_sim=trace_sim) as tc:
        with (
            tc.tile_pool(name="consts", bufs=1) as consts,
            tc.tile_pool(name="acts", bufs=1) as acts,
            tc.tile_pool(name="gbuf", bufs=6) as gpool,
            tc.tile_pool(name="selp", bufs=3) as selpool,
            tc.tile_pool(name="scratch", bufs=7) as scratch,
        ):
          with (
            tc.tile_pool(name="pagg", bufs=1, space="PSUM") as pagg,
          ):
              # ---- gather-critical loads first ------------------------------
              idx16_all = consts.tile([P, T_total * 8], mybir.dt.int16,
                                      tag="idx16a")
              nc.sync.dma_start(out=idx16_all[:], in_=idx16_d[:])
              grpc = consts.tile([P, E_pad_total], f16, tag="grpc")
              iota = consts.tile([P, K_SEL * W_SEL], f16, tag="iota")
              nc.sync.dma_start(out=grpc[:], in_=grp_d[:])
              nc.sync.dma_start(out=iota[:], in_=iota_d[:])

              # warm-up: a dummy 128-idx gather pays the ~6us ext-isa IRAM
              # load + first-call overhead while idx16 is still in flight
              widx = consts.tile([P, 8], mybir.dt.int16, tag="widx")
              wout = consts.tile([P, D], f16, tag="wout")
              nc.vector.memset(widx[:], 0)
              nc.gpsimd.dma_gather(
                  out_ap=wout[:].rearrange("p (c e) -> p c e", c=1),
                  in_ap=x_d[0:BUCKET, :],
                  idxs_ap=widx[:],
                  num_idxs=P,
                  num_idxs_reg=P,
                  elem_size=D,
                  single_packet=SINGLE_PACKET,
                  queue_num=0,
              )

              agg_ps = [pagg.tile([P, 4 * P], f32, tag=f"agg{k}", name=f"agg{k}")
                        for k in range(6)]

              def agg_slice(ch):
                  return agg_ps[ch // 4][:, (ch % 4) * P:(ch % 4 + 1) * P]

              # start=True resets a whole PSUM bank -> only the first matmul
              # touching each bank may set it
              first_pair, last_pair = {}, {}
              pi_count = 0
              for b in range(NBUK):
                  for (t, base, s0, wtot) in entries_by_bucket[b]:
                      bank = base // 512
                      if bank not in first_pair:
                          first_pair[bank] = pi_count
                      last_pair[bank] = pi_count
                      pi_count += 1

              # ---- dispatch bucketed gathers --------------------------------
              # split each bucket into <=MAX_GT-tile calls so every engine's
              # concatenated (single_packet) stream stays <=64 descriptors —
              # the SDMA packet spec ceiling.  Consecutive calls round-robin
              # the 4 SWDGE queues: descgen runs ahead on 4 Q7 core-pairs and
              # 4 rings drain concurrently.
              MAX_GT = 8
              gtiles = {}
              call_idx = 0

              def dispatch_gather(b):
                  nonlocal call_idx
                  cb = Cb[b]
                  g = gpool.tile([P, cb * P], f16, tag="gb", name=f"g{b}")
                  gtiles[b] = g
                  lo = b * BUCKET
                  hi = min(lo + BUCKET, N_NODES)
                  for c0 in range(0, cb, MAX_GT):
                      c1 = min(c0 + MAX_GT, cb)
                      n = c1 - c0
                      g3 = g[:, c0 * D:c1 * D].rearrange("p (c e) -> p c e",
                                                         c=n)
                      nc.gpsimd.dma_gather(
                          out_ap=g3,
                          in_ap=x_d[lo:hi, :],
                          idxs_ap=idx16_all[:, (tile_base[b] + c0) * 8:
                                            (tile_base[b] + c1) * 8],
                          num_idxs=n * P,
                          num_idxs_reg=n * P,
                          elem_size=D,
                          single_packet=SINGLE_PACKET,
                          queue_num=call_idx % NQ,
                      )
                      call_idx += 1

              for b in range(NBUK):
                  dispatch_gather(b)

              # ---- remaining const loads ------------------------------------
              w1a = consts.tile([D, D], f16, tag="w1a")
              w1b = consts.tile([D, D], f16, tag="w1b")
              w2a = consts.tile([D, D], f16, tag="w2a")
              w2b = consts.tile([D, D], f16, tag="w2b")
              b1t = consts.tile([D, 1], f32, tag="b1t")
              b2t = consts.tile([D, 1], f32, tag="b2t")
              ones = consts.tile([P, 1], f16, tag="ones")
              ones32 = consts.tile([1, P], f32, tag="ones32")
              eps = consts.tile([P, 1], f32, tag="eps")
              nc.vector.memset(eps[:], 1e-6)
              nc.vector.memset(ones[:], 1.0)
              nc.vector.memset(ones32[:], 1.0)
              for dst, srcd in ((w1a, w1a_d), (w1b, w1b_d), (w2a, w2a_d),
                                (w2b, w2b_d), (b1t, b1_d), (b2t, b2_d)):
                  nc.sync.dma_start(out=dst[:], in_=srcd[:])

              # ---- batched sel builds + f16 aggregation matmuls -------------
              # sel layout [row_p, entry_e, window_w]: window contiguous per
              # entry so the PE streams 1 col/cycle (a strided rhs costs ~4x).
              pi = 0
              for b in range(NBUK):
                  ents = entries_by_bucket[b]
                  g = gtiles[b]
                  for c0 in range(0, Eb[b], K_SEL):
                      k = min(K_SEL, Eb[b] - c0)
                      chunk = ents[c0:c0 + k]
                      e_col = int(bucket_ebase[b]) + c0
                      sel = selpool.tile([P, K_SEL * W_SEL], f16, tag="sel",
                                         name=f"sel{b}_{c0}")
                      sel3 = sel[:].rearrange("p (e w) -> p e w", e=K_SEL)
                      nc.vector.tensor_tensor(
                          out=sel3[:, :k, :],
                          in0=grpc[:, e_col:e_col + k]
                              .broadcast_to([P, k, W_SEL]),
                          in1=iota[:].rearrange("p (e w) -> p e w",
                                                e=K_SEL)[:, :k, :],
                          op=mybir.AluOpType.is_equal,
                      )
                      for ke, (t, base, s0, wtot) in enumerate(chunk):
                          tl = t - tile_base[b]
                          gt = g[:, tl * D:(tl + 1) * D]
                          bank = base // 512
                          off = base % 512
                          nc.tensor.matmul(
                              out=agg_ps[bank][:, off + s0:off + wtot],
                              lhsT=gt,
                              rhs=sel3[:, ke, s0:wtot],
                              start=(first_pair[bank] == pi),
                              stop=(last_pair[bank] == pi),
                              skip_group_check=True,
                          )
                          pi += 1

              # ---- copy aggregated sums PSUM -> SBUF (fp16) -----------------
              agg_sb = []
              for ch in range(NCHAIN):
                  a = acts.tile([D, B_LOC], f16, tag=f"aggT{ch}",
                                name=f"aggT{ch}")
                  nc.scalar.copy(out=a[:], in_=agg_slice(ch))
                  agg_sb.append(a)

          # ---- SAGE layer in transposed fp16 layout (agg banks now free) --
          with tc.tile_pool(name="psage", bufs=8, space="PSUM") as psage:
            h1n_all = acts.tile([P, S2 * P], f16, tag="h1n_all")

            def sage_group(specs):
                """Stage-major emission of several independent SAGE heads so
                the engines pipeline across them."""
                phs, hs, h2s, psss, nvs, nrs, pbcs = \
                    [], [], [], [], [], [], []
                for i, (rs, ra, wa, wb, bt, tagn, hn) in enumerate(specs):
                    ph = psage.tile([P, P], f32, tag="ps", name=f"ph_{tagn}")
                    nc.tensor.matmul(out=ph[:], lhsT=wa[:], rhs=rs,
                                     start=True, stop=False,
                                     skip_group_check=True)
                    nc.tensor.matmul(out=ph[:], lhsT=wb[:], rhs=ra,
                                     start=False, stop=True,
                                     skip_group_check=True)
                    phs.append(ph)
                for i, (rs, ra, wa, wb, bt, tagn, hn) in enumerate(specs):
                    h = scratch.tile([P, P], f16, tag="h", name=f"h_{tagn}")
                    nc.scalar.activation(
                        out=h[:], in_=phs[i][:],
                        func=mybir.ActivationFunctionType.Relu,
                        bias=bt[:, :1])
                    hs.append(h)
                for i, (rs, ra, wa, wb, bt, tagn, hn) in enumerate(specs):
                    h2 = scratch.tile([P, P], f16, tag="h2", name=f"h2_{tagn}")
                    nc.scalar.square(out=h2[:], in_=hs[i][:])
                    h2s.append(h2)
                for i, (rs, ra, wa, wb, bt, tagn, hn) in enumerate(specs):
                    pss = psage.tile([P, P], f32, tag="ps", name=f"pss_{tagn}")
                    nc.tensor.matmul(out=pss[:1, :], lhsT=ones[:, :1],
                                     rhs=h2s[i][:], start=True, stop=True,
                                     skip_group_check=True)
                    psss.append(pss)
                for i, (rs, ra, wa, wb, bt, tagn, hn) in enumerate(specs):
                    nv = scratch.tile([P, P], f32, tag="nv", name=f"nv_{tagn}")
                    nc.scalar.activation(
                        out=nv[:1, :], in_=psss[i][:1, :],
                        func=mybir.ActivationFunctionType.Sqrt,
                        bias=eps[:1, :1])
                    nvs.append(nv)
                for i, (rs, ra, wa, wb, bt, tagn, hn) in enumerate(specs):
                    nr = scratch.tile([P, P], f32, tag="nr", name=f"nr_{tagn}")
                    nc.vector.reciprocal_approx_fast(out=nr[:1, :],
                                                     in_=nvs[i][:1, :])
                    nrs.append(nr)
                for i, (rs, ra, wa, wb, bt, tagn, hn) in enumerate(specs):
                    pbc = psage.tile([P, P], f32, tag="ps", name=f"pbc_{tagn}")
                    nc.tensor.matmul(out=pbc[:], lhsT=ones32[:1, :],
                                     rhs=nrs[i][:1, :], start=True, stop=True,
                                     skip_group_check=True)
                    pbcs.append(pbc)
                outs = []
                for i, (rs, ra, wa, wb, bt, tagn, hn) in enumerate(specs):
                    if hn is None:
                        hn = acts.tile([D, B_LOC], f16, tag=tagn,
                                       name=tagn)[:]
                    nc.vector.tensor_tensor(out=hn, in0=hs[i][:],
                                            in1=pbcs[i][:],
                                            op=mybir.AluOpType.mult)
                    outs.append(hn)
                return outs

            h1n_slice = lambda j: h1n_all[:, j * P:(j + 1) * P]
            self_t = agg_sb[NCHAIN_AGG]
            specs = [(self_t[:], agg_sb[0][:], w1a, w1b, b1t, "h1t", None)]
            specs += [(agg_sb[NCHAIN_AGG + 1 + j][:], agg_sb[1 + j][:],
                       w1a, w1b, b1t, f"h1n{j}", h1n_slice(j))
                      for j in range(S2)]
            res0 = sage_group(specs[:6])
            h1t = res0[0]
            sage_group(specs[6:])

            a3 = acts.tile([D, B_LOC], f16, tag="a3")
            with nc.allow_low_precision("fp16 sum of 10 normalized values"):
                nc.vector.reduce_sum(
                    out=a3[:],
                    in_=h1n_all[:].rearrange("p (j r) -> p r j", j=S2),
                    axis=mybir.AxisListType.X,
                )

            zt = sage_group([(h1t, a3[:], w2a, w2b, b2t, "zt", None)])[0]
            nc.sync.dma_start(out=zt_d[:], in_=zt)

    nc.finalize()
    return nc


def kernel(x, W1, b1, W2, b2, nodes, nb2, nb1_self, nb1_nb,
           _trace=False, _core_ids=None):
    x16 = np.ascontiguousarray(np.asarray(x, dtype=np.float16))
    W1 = np.asarray(W1, dtype=np.float32)
    W2 = np.asarray(W2, dtype=np.float32)
    b1 = np.asarray(b1, dtype=np.float32)
    b2 = np.asarray(b2, dtype=np.float32)
    nodes = np.asarray(nodes)
    nb2 = np.asarray(nb2)
    nb1_self = np.asarray(nb1_self)
    nb1_nb = np.asarray(nb1_nb)

    meta = _prep_indices(nodes, nb2, nb1_self, nb1_nb)
    nc = _build_program(meta)

    # host-prescaled weights: the 1/25 and 1/10 means fold into W*b
    w1a = np.ascontiguousarray(W1[:D]).astype(np.float16)
    w1b = np.ascontiguousarray(W1[D:] / S1).astype(np.float16)
    w2a = np.ascontiguousarray(W2[:D]).astype(np.float16)
    w2b = np.ascontiguousarray(W2[D:] / S2).astype(np.float16)
    # iota[p, e*W_SEL + w] = w  (window contiguous per entry)
    iota = np.tile(np.arange(W_SEL, dtype=np.float16), K_SEL)
    iota = np.ascontiguousarray(np.broadcast_to(iota, (P, K_SEL * W_SEL)))

    in_maps = []
    for c in range(NCORES):
        in_maps.append({
            "x16": x16,
            "w1a": w1a, "w1b": w1b, "w2a": w2a, "w2b": w2b,
            "b1v": b1.reshape(D, 1), "b2v": b2.reshape(D, 1),
            "iota": iota,
            "idx16": meta["idx16_cores"][c], "grp": meta["grp_ent_cores"][c],
        })

    from concourse.bass_utils import run_bass_kernel_spmd

    core_ids = _core_ids if _core_ids is not None else list(range(NCORES))
    res = run_bass_kernel_spmd(nc, in_maps[:len(core_ids)], core_ids=core_ids,
                               trace=_trace)
    z = np.concatenate([res.results[c]["zt"].T.astype(np.float32)
                        for c in range(len(core_ids))], axis=0)
    kernel.last_exec_time_ns = res.exec_time_ns
    kernel.last_results = res
    return z


# revision 28
# speedup vs baseline: 1.0807x; 1.0807x over previous
"""GraphSAGE 2-layer mini-batch kernel for 8 Trainium2 NeuronCores (v3).

Strategy: data-parallel over the batch (128 targets per core); the feature
table is uploaded as fp16 (halves gather bytes; tolerance is 2e-2).

ALL 36,608 rows per core (nb1_self, nb1_nb, nodes, nb2) go through the Q7
dma_gather path: indices bucket-sorted into 16 buckets of 32768 rows
(int16-addressable), one dma_gather per bucket over 4 SWDGE queues.
nodes/nb2 rows are singleton-group "chains" 11..21 so the same
selection-matmul machinery that undoes the bucket permutation also lands
them transposed [feature, row] — no separate indirect DMAs / PE transposes.

v3 changes vs v2:
- sel matrices are built in [row, window, entry] layout (entry innermost,
  step-1 on every operand) so the DVE runs in 2x_1p packed mode — the v2
  layout broadcast grp along the innermost axis which forced 1x.
- the agg matmuls read sel with a strided rhs AP (entry-stride columns).
- one dma_gather per bucket (16 calls instead of 20) to cut fixed SWDGE
  overhead; descriptor generation on the Q7 is the serial bottleneck.
- the whole SAGE phase runs in fp16 (weights, activations, norms): fp16
  matmuls stream 1 col/cycle vs fp32r's 4-cycle penalty, and the final
  normalize multiply runs packed.
- the per-head reciprocal runs on the tiny [1,128] norm vector BEFORE the
  rank-1 broadcast matmul (v2 reciprocal'd the broadcast [128,128]).

The 1/25 and 1/10 mean scalings are folded into host-prescaled W1/W2.
L2 norms use a ones-vector matmul for the cross-partition reduction.
"""
import sys

sys.path.insert(0, "/opt/trn_rl_repo")

import numpy as np

P = 128
D = 128
B = 1024
S1 = 25
S2 = 10
N_NODES = 500000
NCORES = 8
B_LOC = B // NCORES          # 128 targets per core
NCHAIN_AGG = 11              # nb1_self + 10 nb1_nb chains (mean groups)
NCHAIN = 22                  # + nodes chain + 10 nb2 chains (singletons)
BUCKET_BITS = 15
BUCKET = 1 << BUCKET_BITS    # 32768 rows per bucket (int16 addressable)
NBUK = (N_NODES + BUCKET - 1) // BUCKET  # 16
NQ = 4                       # SWDGE queues: 4 rings = 4x descgen core-pairs
                             # and 4x outstanding SDMA descriptors
W_SEL = 128                  # sel window width (covers a tile's group span)
K_SEL = 16                   # sel entries built per DVE op
NGRP = NCHAIN * B_LOC        # 2816 global groups
SINGLE_PACKET = True         # concatenate each gather's descs per engine:
                             # amortizes per-packet SDMA overhead/latency
SCRATCH_SIZE = 131072       # SWDGE descriptor carveout (per-partition bytes)


def _prep_indices(nodes, nb2, nb1_self, nb1_nb):
    """Bucket-sort all gather indices per core; build device-side arrays and
    the (core-independent) per-entry metadata.

    Entry = (tile, base_chain): a 256-wide sel window covering chains
    {base, base+1}.  Tiles spanning more than 2 chains get several entries.
    """
    per_core = []
    for c in range(NCORES):
        sl = slice(c * B_LOC, (c + 1) * B_LOC)
        n1s = nb1_self[sl]              # [128, 25]
        n1n = nb1_nb[sl]                # [128, 10, 25]
        idx_chains = [n1s.reshape(-1).astype(np.int64)]
        grp_chains = [np.repeat(np.arange(B_LOC, dtype=np.int64), S1)]
        for j in range(S2):
            idx_chains.append(n1n[:, j, :].reshape(-1).astype(np.int64))
            grp_chains.append((j + 1) * B_LOC
                              + np.repeat(np.arange(B_LOC, dtype=np.int64), S1))
        # singleton chains: 11 = nodes, 12+j = nb2[:, j]
        idx_chains.append(nodes[sl].astype(np.int64))
        grp_chains.append(NCHAIN_AGG * B_LOC + np.arange(B_LOC, dtype=np.int64))
        for j in range(S2):
            idx_chains.append(nb2[sl, j].astype(np.int64))
            grp_chains.append((NCHAIN_AGG + 1 + j) * B_LOC
                              + np.arange(B_LOC, dtype=np.int64))
        all_idx = np.concatenate(idx_chains)   # [36608]
        all_grp = np.concatenate(grp_chains)
        bkt = all_idx >> BUCKET_BITS
        order = np.argsort(bkt, kind="stable")
        sidx, sgrp, sbkt = all_idx[order], all_grp[order], bkt[order]
        locs, grps = [], []
        for b in range(NBUK):
            m = sbkt == b
            locs.append((sidx[m] - (b << BUCKET_BITS)).astype(np.int64))
            grps.append(sgrp[m])
        per_core.append((locs, grps))

    # consistent per-bucket tile counts across cores (SPMD: one program)
    Cb = [max((len(per_core[c][0][b]) + P - 1) // P for c in range(NCORES))
          for b in range(NBUK)]
    Cb = [max(cb, 1) for cb in Cb]
    T_total = sum(Cb)

    S_total = T_total * P
    # Padding rows: buckets 0-5 pad with row 0 (their SBUF slots are fresh —
    # unwritten garbage could be Inf/NaN and poison the 0-weighted matmul);
    # buckets 6+ pad with -1 (trailing negatives skip descgen AND drain; the
    # recycled slot holds finite fp16 from an earlier bucket).
    idx16_cores, grp_glob = [], []
    for c in range(NCORES):
        locs, grps = per_core[c]
        lidx = np.zeros(S_total, np.int64)
        lgrp = np.full(S_total, -1024.0, np.float64)
        off = 0
        for b in range(NBUK):
            n = len(locs[b])
            lidx[off:off + n] = locs[b]
            lgrp[off:off + n] = grps[b]
            off += Cb[b] * P
        wrapped_cols = []
        off = 0
        for b in range(NBUK):
            nb_pad = Cb[b] * P
            w = lidx[off:off + nb_pad].reshape(-1, 16).T.astype(np.int16)
            wrapped_cols.append(np.tile(w, (8, 1)))
            off += nb_pad
        idx16_cores.append(np.hstack(wrapped_cols))        # [128, T_total*8]
        grp_glob.append(lgrp.reshape(T_total, P).T)        # [128, T_total]

    # per-tile GROUP spans (global group ids are sorted within a bucket, so a
    # 128-row tile covers a narrow contiguous group range), unioned across
    # cores so metadata is SPMD-safe
    tile_span = []
    for t in range(T_total):
        glo, ghi = None, None
        for c in range(NCORES):
            g = grp_glob[c][:, t]
            v = g[g >= 0]
            if v.size:
                l, h = int(v.min()), int(v.max())
                glo = l if glo is None else min(glo, l)
                ghi = h if ghi is None else max(ghi, h)
        tile_span.append((glo, ghi))

    # entries: (tile, base, s0, wtot) — an unaligned W_SEL-wide sel window at
    # global-group offset `base`.  `base` is nudged down so the window never
    # crosses a 512-col PSUM bank boundary (single matmul per entry); the
    # matmul streams sel columns [s0, wtot) so overlapping windows of the
    # same tile never double-count a row.
    def pick_base(lo):
        base = lo
        if base % 512 > 512 - W_SEL:
            base = (base // 512) * 512 + (512 - W_SEL)
        return base

    tile_base = np.cumsum([0] + Cb)
    entries_by_bucket = []
    for b in range(NBUK):
        ents = []
        for tl in range(Cb[b]):
            t = tile_base[b] + tl
            glo, ghi = tile_span[t]
            if glo is None:
                continue
            nxt = glo
            first = True
            while True:
                base = pick_base(nxt)
                s0 = 0 if first else nxt - base
                wtot = min(W_SEL, NGRP - base)
                ents.append((t, base, s0, wtot))
                first = False
                nxt = base + W_SEL
                if nxt > ghi:
                    break
        entries_by_bucket.append(ents)

    # per-core grp data laid out per ENTRY, values local to the entry window.
    # Each bucket's entry-column block is padded to an even count so every
    # K_SEL-aligned chunk slice starts 4B-aligned (2x_1p DVE mode).
    Eb = [len(e) for e in entries_by_bucket]
    Eb_pad = [e + (e % 2) for e in Eb]
    bucket_ebase = np.cumsum([0] + Eb_pad)
    E_pad_total = int(bucket_ebase[-1])
    grp_ent_cores = []
    for c in range(NCORES):
        ge = np.full((P, E_pad_total), -2048.0, np.float16)
        for b in range(NBUK):
            for j, (t, base, s0, wtot) in enumerate(entries_by_bucket[b]):
                # local offsets; pad rows (-1024 global) clamp to -2048 so the
                # fp16 value stays integer-exact and never matches iota
                loc = grp_glob[c][:, t] - base
                loc[loc < -2048] = -2048
                ge[:, bucket_ebase[b] + j] = loc.astype(np.float16)
        grp_ent_cores.append(ge)

    return dict(Cb=Cb, T_total=T_total, tile_base=tile_base,
                entries_by_bucket=entries_by_bucket, Eb=Eb,
                bucket_ebase=bucket_ebase, E_pad_total=E_pad_total,
                idx16_cores=idx16_cores, grp_ent_cores=grp_ent_cores)


def _build_program(meta, trace_sim=False):
    import concourse.bacc as bacc_mod
    import concourse.tile as tile
    from concourse import mybir

    f32 = mybir.dt.float32
    f16 = mybir.dt.float16
    Cb = meta["Cb"]
    T_total = meta["T_total"]
    tile_base = meta["tile_base"]
    entries_by_bucket = meta["entries_by_bucket"]
    Eb = meta["Eb"]
    bucket_ebase = meta["bucket_ebase"]
    E_pad_total = meta["E_pad_total"]

    nc = bacc_mod.Bacc(num_swdge_queues=NQ,
                       dynamic_dma_scratch_size=SCRATCH_SIZE)

    x_d = nc.declare_dram_parameter("x16", [N_NODES, D], f16, isOutput=False)
    w1a_d = nc.declare_dram_parameter("w1a", [D, D], f16, isOutput=False)
    w1b_d = nc.declare_dram_parameter("w1b", [D, D], f16, isOutput=False)
    w2a_d = nc.declare_dram_parameter("w2a", [D, D], f16, isOutput=False)
    w2b_d = nc.declare_dram_parameter("w2b", [D, D], f16, isOutput=False)
    b1_d = nc.declare_dram_parameter("b1v", [D, 1], f32, isOutput=False)
    b2_d = nc.declare_dram_parameter("b2v", [D, 1], f32, isOutput=False)
    iota_d = nc.declare_dram_parameter("iota", [P, W_SEL * K_SEL], f16,
                                       isOutput=False)
    idx16_d = nc.declare_dram_parameter("idx16", [P, T_total * 8],
                                        mybir.dt.int16, isOutput=False)
    grp_d = nc.declare_dram_parameter("grp", [P, E_pad_total], f16,
                                      isOutput=False)
    zt_d = nc.declare_dram_parameter("zt", [D, B_LOC], f16, isOutput=True)

    with tile.TileContext(nc, trace_sim=trace_sim) as tc:
        with (
            tc.tile_pool(name="consts", bufs=1) as consts,
            tc.tile_pool(name="acts", bufs=1) as acts,
            tc.tile_pool(name="gbuf", bufs=6) as gpool,
            tc.tile_pool(name="selp", bufs=4) as selpool,
            tc.tile_pool(name="scratch", bufs=7) as scratch,
        ):
          with (
            tc.tile_pool(name="pagg", bufs=1, space="PSUM") as pagg,
          ):
              # ---- gather-critical loads first ------------------------------
              idx16_all = consts.tile([P, T_total * 8], mybir.dt.int16,
                                      tag="idx16a")
              nc.sync.dma_start(out=idx16_all[:], in_=idx16_d[:])
              grpc = consts.tile([P, E_pad_total], f16, tag="grpc")
              iota = consts.tile([P, K_SEL * W_SEL], f16, tag="iota")
              nc.sync.dma_start(out=grpc[:], in_=grp_d[:])
              nc.sync.dma_start(out=iota[:], in_=iota_d[:])

              # warm-up: a dummy 128-idx gather pays the ~6us ext-isa IRAM
              # load + first-call overhead while idx16 is still in flight
              widx = consts.tile([P, 8], mybir.dt.int16, tag="widx")
              wout = consts.tile([P, D], f16, tag="wout")
              nc.vector.memset(widx[:], 0)
              nc.gpsimd.dma_gather(
                  out_ap=wout[:].rearrange("p (c e) -> p c e", c=1),
                  in_ap=x_d[0:BUCKET, :],
                  idxs_ap=widx[:],
                  num_idxs=P,
                  num_idxs_reg=P,
                  elem_size=D,
                  single_packet=SINGLE_PACKET,
                  queue_num=0,
              )

              agg_ps = [pagg.tile([P, 4 * P], f32, tag=f"agg{k}", name=f"agg{k}")
                        for k in range(6)]

              def agg_slice(ch):
                  return agg_ps[ch // 4][:, (ch % 4) * P:(ch % 4 + 1) * P]

              # start=True resets a whole PSUM bank -> only the first matmul
              # touching each bank may set it
              first_pair, last_pair = {}, {}
              pi_count = 0
              for b in range(NBUK):
                  for (t, base, s0, wtot) in entries_by_bucket[b]:
                      bank = base // 512
                      if bank not in first_pair:
                          first_pair[bank] = pi_count
                      last_pair[bank] = pi_count
                      pi_count += 1

              # ---- dispatch bucketed gathers --------------------------------
              # split each bucket into <=MAX_GT-tile calls so every engine's
              # concatenated (single_packet) stream stays <=64 descriptors —
              # the SDMA packet spec ceiling.  Consecutive calls round-robin
              # the 4 SWDGE queues: descgen runs ahead on 4 Q7 core-pairs and
              # 4 rings drain concurrently.
              MAX_GT = 8
              gtiles = {}
              call_idx = 0

              def dispatch_gather(b):
                  nonlocal call_idx
                  cb = Cb[b]
                  g = gpool.tile([P, cb * P], f16, tag="gb", name=f"g{b}")
                  gtiles[b] = g
                  lo = b * BUCKET
                  hi = min(lo + BUCKET, N_NODES)
                  for c0 in range(0, cb, MAX_GT):
                      c1 = min(c0 + MAX_GT, cb)
                      n = c1 - c0
                      g3 = g[:, c0 * D:c1 * D].rearrange("p (c e) -> p c e",
                                                         c=n)
                      nc.gpsimd.dma_gather(
                          out_ap=g3,
                          in_ap=x_d[lo:hi, :],
                          idxs_ap=idx16_all[:, (tile_base[b] + c0) * 8:
                                            (tile_base[b] + c1) * 8],
                          num_idxs=n * P,
                          num_idxs_reg=n * P,
                          elem_size=D,
                          single_packet=SINGLE_PACKET,
                          queue_num=call_idx % NQ,
                      )
                      call_idx += 1

              for b in range(NBUK):
                  dispatch_gather(b)

              # ---- remaining const loads ------------------------------------
              w1a = consts.tile([D, D], f16, tag="w1a")
              w1b = consts.tile([D, D], f16, tag="w1b")
              w2a = consts.tile([D, D], f16, tag="w2a")
              w2b = consts.tile([D, D], f16, tag="w2b")
              b1t = consts.tile([D, 1], f32, tag="b1t")
              b2t = consts.tile([D, 1], f32, tag="b2t")
              ones = consts.tile([P, 1], f16, tag="ones")
              ones32 = consts.tile([1, P], f32, tag="ones32")
              eps = consts.tile([P, 1], f32, tag="eps")
              nc.vector.memset(eps[:], 1e-6)
              nc.vector.memset(ones[:], 1.0)
              nc.vector.memset(ones32[:], 1.0)
              for dst, srcd in ((w1a, w1a_d), (w1b, w1b_d), (w2a, w2a_d),
                                (w2b, w2b_d), (b1t, b1_d), (b2t, b2_d)):
                  nc.sync.dma_start(out=dst[:], in_=srcd[:])

              # ---- batched sel builds + f16 aggregation matmuls -------------
              # sel layout [row_p, entry_e, window_w]: window contiguous per
              # entry so the PE streams 1 col/cycle (a strided rhs costs ~4x).
              pi = 0
              for b in range(NBUK):
                  ents = entries_by_bucket[b]
                  g = gtiles[b]
                  for c0 in range(0, Eb[b], K_SEL):
                      k = min(K_SEL, Eb[b] - c0)
                      chunk = ents[c0:c0 + k]
                      e_col = int(bucket_ebase[b]) + c0
                      sel = selpool.tile([P, K_SEL * W_SEL], f16, tag="sel",
                                         name=f"sel{b}_{c0}")
                      sel3 = sel[:].rearrange("p (e w) -> p e w", e=K_SEL)
                      nc.vector.tensor_tensor(
                          out=sel3[:, :k, :],
                          in0=grpc[:, e_col:e_col + k]
                              .broadcast_to([P, k, W_SEL]),
                          in1=iota[:].rearrange("p (e w) -> p e w",
                                                e=K_SEL)[:, :k, :],
                          op=mybir.AluOpType.is_equal,
                      )
                      for ke, (t, base, s0, wtot) in enumerate(chunk):
                          tl = t - tile_base[b]
                          gt = g[:, tl * D:(tl + 1) * D]
                          bank = base // 512
                          off = base % 512
                          nc.tensor.matmul(
                              out=agg_ps[bank][:, off + s0:off + wtot],
                              lhsT=gt,
                              rhs=sel3[:, ke, s0:wtot],
                              start=(first_pair[bank] == pi),
                              stop=(last_pair[bank] == pi),
                              skip_group_check=True,
                          )
                          pi += 1

              # ---- copy aggregated sums PSUM -> SBUF (fp16) -----------------
              agg_sb = []
              for ch in range(NCHAIN):
                  a = acts.tile([D, B_LOC], f16, tag=f"aggT{ch}",
                                name=f"aggT{ch}")
                  nc.scalar.copy(out=a[:], in_=agg_slice(ch))
                  agg_sb.append(a)

          # ---- SAGE layer in transposed fp16 layout (agg banks now free) --
          with tc.tile_pool(name="psage", bufs=8, space="PSUM") as psage:
            h1n_all = acts.tile([P, S2 * P], f16, tag="h1n_all")

            def sage_group(specs):
                """Stage-major emission of several independent SAGE heads so
                the engines pipeline across them."""
                phs, hs, h2s, psss, nvs, nrs, pbcs = \
                    [], [], [], [], [], [], []
                for i, (rs, ra, wa, wb, bt, tagn, hn) in enumerate(specs):
                    ph = psage.tile([P, P], f32, tag="ps", name=f"ph_{tagn}")
                    nc.tensor.matmul(out=ph[:], lhsT=wa[:], rhs=rs,
                                     start=True, stop=False,
                                     skip_group_check=True)
                    nc.tensor.matmul(out=ph[:], lhsT=wb[:], rhs=ra,
                                     start=False, stop=True,
                                     skip_group_check=True)
                    phs.append(ph)
                for i, (rs, ra, wa, wb, bt, tagn, hn) in enumerate(specs):
                    h = scratch.tile([P, P], f16, tag="h", name=f"h_{tagn}")
                    nc.scalar.activation(
                        out=h[:], in_=phs[i][:],
                        func=mybir.ActivationFunctionType.Relu,
                        bias=bt[:, :1])
                    hs.append(h)
                for i, (rs, ra, wa, wb, bt, tagn, hn) in enumerate(specs):
                    h2 = scratch.tile([P, P], f16, tag="h2", name=f"h2_{tagn}")
                    nc.scalar.square(out=h2[:], in_=hs[i][:])
                    h2s.append(h2)
                for i, (rs, ra, wa, wb, bt, tagn, hn) in enumerate(specs):
                    pss = psage.tile([P, P], f32, tag="ps", name=f"pss_{tagn}")
                    nc.tensor.matmul(out=pss[:1, :], lhsT=ones[:, :1],
                                     rhs=h2s[i][:], start=True, stop=True,
                                     skip_group_check=True)
                    psss.append(pss)
                for i, (rs, ra, wa, wb, bt, tagn, hn) in enumerate(specs):
                    nv = scratch.tile([P, P], f32, tag="nv", name=f"nv_{tagn}")
                    nc.scalar.activation(
                        out=nv[:1, :], in_=psss[i][:1, :],
                        func=mybir.ActivationFunctionType.Sqrt,
                        bias=eps[:1, :1])
                    nvs.append(nv)
                for i, (rs, ra, wa, wb, bt, tagn, hn) in enumerate(specs):
                    nr = scratch.tile([P, P], f32, tag="nr", name=f"nr_{tagn}")
                    nc.vector.reciprocal_approx_fast(out=nr[:1, :],
                                                     in_=nvs[i][:1, :])
                    nrs.append(nr)
                for i, (rs, ra, wa, wb, bt, tagn, hn) in enumerate(specs):
                    pbc = psage.tile([P, P], f32, tag="ps", name=f"pbc_{tagn}")
                    nc.tensor.matmul(out=pbc[:], lhsT=ones32[:1, :],
                                     rhs=nrs[i][:1, :], start=True, stop=True,
                                     skip_group_check=True)
                    pbcs.append(pbc)
                outs = []
                for i, (rs, ra, wa, wb, bt, tagn, hn) in enumerate(specs):
                    if hn is None:
                        hn = acts.tile([D, B_LOC], f16, tag=tagn,
                                       name=tagn)[:]
                    nc.vector.tensor_tensor(out=hn, in0=hs[i][:],
                                            in1=pbcs[i][:],
                                            op=mybir.AluOpType.mult)
                    outs.append(hn)
                return outs

            h1n_slice = lambda j: h1n_all[:, j * P:(j + 1) * P]
            self_t = agg_sb[NCHAIN_AGG]
            specs = [(self_t[:], agg_sb[0][:], w1a, w1b, b1t, "h1t", None)]
            specs += [(agg_sb[NCHAIN_AGG + 1 + j][:], agg_sb[1 + j][:],
                       w1a, w1b, b1t, f"h1n{j}", h1n_slice(j))
                      for j in range(S2)]
            res0 = sage_group(specs[:6])
            h1t = res0[0]
            sage_group(specs[6:])

            a3 = acts.tile([D, B_LOC], f16, tag="a3")
            with nc.allow_low_precision("fp16 sum of 10 normalized values"):
                nc.vector.reduce_sum(
                    out=a3[:],
                    in_=h1n_all[:].rearrange("p (j r) -> p r j", j=S2),
                    axis=mybir.AxisListType.X,
                )

            zt = sage_group([(h1t, a3[:], w2a, w2b, b2t, "zt", None)])[0]
            nc.sync.dma_start(out=zt_d[:], in_=zt)

    nc.finalize()
    return nc


def kernel(x, W1, b1, W2, b2, nodes, nb2, nb1_self, nb1_nb,
           _trace=False, _core_ids=None):
    x16 = np.ascontiguousarray(np.asarray(x, dtype=np.float16))
    W1 = np.asarray(W1, dtype=np.float32)
    W2 = np.asarray(W2, dtype=np.float32)
    b1 = np.asarray(b1, dtype=np.float32)
    b2 = np.asarray(b2, dtype=np.float32)
    nodes = np.asarray(nodes)
    nb2 = np.asarray(nb2)
    nb1_self = np.asarray(nb1_self)
    nb1_nb = np.asarray(nb1_nb)

    meta = _prep_indices(nodes, nb2, nb1_self, nb1_nb)
    nc = _build_program(meta)

    # host-prescaled weights: the 1/25 and 1/10 means fold into W*b
    w1a = np.ascontiguousarray(W1[:D]).astype(np.float16)
    w1b = np.ascontiguousarray(W1[D:] / S1).astype(np.float16)
    w2a = np.ascontiguousarray(W2[:D]).astype(np.float16)
    w2b = np.ascontiguousarray(W2[D:] / S2).astype(np.float16)
    # iota[p, e*W_SEL + w] = w  (window contiguous per entry)
    iota = np.tile(np.arange(W_SEL, dtype=np.float16), K_SEL)
    iota = np.ascontiguousarray(np.broadcast_to(iota, (P, K_SEL * W_SEL)))

    in_maps = []
    for c in range(NCORES):
        in_maps.append({
            "x16": x16,
            "w1a": w1a, "w1b": w1b, "w2a": w2a, "w2b": w2b,
            "b1v": b1.reshape(D, 1), "b2v": b2.reshape(D, 1),
            "iota": iota,
            "idx16": meta["idx16_cores"][c], "grp": meta["grp_ent_cores"][c],
        })

    from concourse.bass_utils import run_bass_kernel_spmd

    core_ids = _core_ids if _core_ids is not None else list(range(NCORES))
    res = run_bass_kernel_spmd(nc, in_maps[:len(core_ids)], core_ids=core_ids,
                               trace=_trace)
    z = np.concatenate([res.results[c]["zt"].T.astype(np.float32)
                        for c in range(len(core_ids))], axis=0)
    kernel.last_exec_time_ns = res.exec_time_ns
    kernel.last_results = res
    return z
